# revision 30
# baseline (speedup 1.0000x reference)
"""Tensor-parallel causal MHA kernel for 8 Trainium2 NeuronCores.

Problem: B=4, L=2048, D=1024, H=16 heads (hd=64), f32, causal softmax.

Sharding: batch-DP x head-TP. Core c handles batch b=c//2 and head group
g=c%2 (8 heads = 512 feature dims). Each core computes its QKV column
shard, attention for its 8 heads over its batch, and a row-shard partial
O-projection. Host sums the two partials per batch and adds bo.

Per-core pipeline (feature-transposed layout to avoid all transposes of
intermediates):
  x^T (PE transpose)  ->  Q^T/K^T = Wq/Wk^T chunks @ x^T  (feature-major,
                          stored bf16; Wq is pre-scaled by 1/sqrt(hd))
                          V       = x^T chunks @ Wv       (token-major,
                                    with a ones column per head for the
                                    softmax denominator)
  scores^T[k,q] = K^T.T @ Q^T  per head-pair as adjacent K=64 bf16
                  matmuls at partition offsets 0/64 (disjoint PE row
                  groups -> they dispatch concurrently on HW)
  diagonal key blocks: only the 128-wide triangular window needs masking
                  (bf16 tri mask add, N=128); columns left of the window
                  are never computed, columns right of it are mask-free
  att = exp(scores^T)  (no max subtraction: scores are O(5) bounded)
  AV^T[d+1,q] = V'.T @ att  accumulated over key blocks; row d is the
                softmax denominator l[q]
  normalize: att_out^T = AV^T[0:64] * broadcast(1/l); both heads of a
                pair share one PE ones-outer-product broadcast; DVE
                writes attoT (bf16) directly
  out[t,o] += att_out^T chunks.T @ Wo chunks  (bf16; partial, host adds
                the core pairs)
  the O-projection is deferred into the next slab's score stream so PE
  never stalls on the last head's normalize chain.

Weights stream in on the gpsimd SWDGE queue, x/out on the SP queue, so
phase A starts ~1.6us in instead of waiting for 19us of weight DMAs.

QKV projections run in fp32r (12-bit mantissa, full PE rate; the walrus
verifier requires fp32r matmul inputs to come from rounding producers,
so x^T tiles are written by ACT with an fp32r view). The attention
score/AV path and the O-projection run in bf16 (same PE rate, full rate
even for narrow free dims, fp32 PSUM accumulation).
"""

import sys

if "/opt/trn_rl_repo" not in sys.path:
    sys.path.insert(0, "/opt/trn_rl_repo")

import ml_dtypes
import numpy as np

import concourse.bass as bass
import concourse.tile as tile
from concourse import bacc, mybir
from concourse.bass_utils import run_bass_kernel_spmd
from concourse.masks import make_identity

F32 = mybir.dt.float32
F32R = mybir.dt.float32r
BF16 = mybir.dt.bfloat16
EXP = mybir.ActivationFunctionType.Exp
IDENT = mybir.ActivationFunctionType.Identity
COPY = mybir.ActivationFunctionType.Copy

N_CORES = 8


def round_f32r(a: np.ndarray) -> np.ndarray:
    """Round fp32 values to the fp32r grid (12-bit mantissa, round half up)."""
    u = np.ascontiguousarray(a, dtype=np.float32).view(np.uint32)
    r = ((u.astype(np.uint64) + 0x800) & 0xFFFFF000).astype(np.uint32)
    return r.view(np.float32)


def build_program(L=2048, D=1024, HPC=8, hd=64, repeat=1, with_bias=True,
                  w_engine="gpsimd", flush_w=4, oproj_defer=True):
    """Build the per-core SPMD Bass program. Returns the compiled Bacc.

    repeat>1 duplicates the whole pipeline (for timing: the wall-clock delta
    between repeat=K and repeat=1 isolates on-device execution time).
    """
    DQ = HPC * hd                # per-core head dims (columns of the shard)
    SL = 512                     # slab width (queries per attention slab)
    NS = L // SL                 # slabs
    TSUB = SL // 128             # 128-row tiles per slab
    DIN = D // 128               # contraction chunks
    DQT = DQ // 128              # 128-dim tiles of the shard
    HPP = 128 // hd              # heads per 128-dim tile (2)
    VW = hd + 1                  # V' width per head (ones column appended)

    nc = bacc.Bacc("TRN2", target_bir_lowering=False, debug=False)

    x_d = nc.dram_tensor("x", [L, D], F32, kind="ExternalInput")
    wq_d = nc.dram_tensor("wq", [D, DQ], F32R, kind="ExternalInput")  # pre-scaled
    wk_d = nc.dram_tensor("wk", [D, DQ], F32R, kind="ExternalInput")
    wv_d = nc.dram_tensor("wv", [D, DQ], F32R, kind="ExternalInput")
    wo_d = nc.dram_tensor("wo", [DQ, D], BF16, kind="ExternalInput")
    bq_d = nc.dram_tensor("bq", [DQ], F32, kind="ExternalInput")   # pre-scaled
    bk_d = nc.dram_tensor("bk", [DQ], F32, kind="ExternalInput")
    bv_d = nc.dram_tensor("bv", [1, DQ], F32R, kind="ExternalInput")
    # 0/1 keep-mask (both parities) for the 128-wide diagonal window
    trib_d = nc.dram_tensor("trib", [128, 2, 128], BF16, kind="ExternalInput")
    out_d = nc.dram_tensor("out", [L, D], F32, kind="ExternalOutput")

    with tile.TileContext(nc) as tc:
        with (
            tc.tile_pool(name="persist", bufs=1) as persist,
            tc.tile_pool(name="consts", bufs=1) as consts,
        ):
            # persistent activations
            qt = persist.tile([128, DQT, L], BF16, tag="qt")
            kt = persist.tile([128, DQT, L], BF16, tag="kt")
            vt = persist.tile([128, L // 128, HPC, VW], BF16, tag="vt")

            ident = consts.tile([128, 128], F32, tag="ident")
            ones_sc = consts.tile([128, 128], F32, tag="ones_sc")
            # ones2v[p*hd, r] = 1 iff head-parity p owns output row r; the
            # 1/l rows of the two parities live at partitions 0 and hd of
            # rv[], so one K=65 outer product broadcasts both
            ones2v = consts.tile([hd + 1, 128], BF16, tag="ones2v")
            rv = [consts.tile([hd + 1, SL], BF16, tag=f"recipv{i}",
                              name=f"recipv{i}") for i in range(2)]
            bq_sb = consts.tile([128, DQT], F32, tag="bq")
            bk_sb = consts.tile([128, DQT], F32, tag="bk")
            trib_sb = consts.tile([128, HPP, 128], BF16, tag="trib")

            make_identity(nc, ident[:])
            nc.gpsimd.memset(ones_sc[:], 1.0)
            nc.gpsimd.memset(ones2v[:], 0.0)
            for i in range(2):
                nc.gpsimd.memset(rv[i][:], 0.0)
            for p in range(HPP):
                nc.scalar.activation(
                    ones2v[p * hd : p * hd + 1, p * hd : (p + 1) * hd],
                    ones_sc[0:1, 0:hd],
                    COPY,
                )
            nc.scalar.activation(
                vt[:, :, :, hd],
                ones_sc[:].rearrange("p (a b) -> p a b", a=L // 128)[:, :, 0:HPC],
                COPY,
            )
            nc.sync.dma_start(bq_sb[:], bq_d[:].rearrange("(c p) -> p c", p=128))
            nc.sync.dma_start(bk_sb[:], bk_d[:].rearrange("(c p) -> p c", p=128))
            nc.sync.dma_start(trib_sb[:], trib_d[:])
            if with_bias:
                ones_k = consts.tile([1, 128], F32R, tag="ones_k")
                bv_sb = consts.tile([1, DQ], F32R, tag="bv")
                nc.scalar.activation(ones_k[:], ones_sc[0:1, :], COPY)
                nc.sync.dma_start(bv_sb[:], bv_d[:])

            def one_pass():
              with tc.tile_pool(name="wob", bufs=1) as wob_pool:
                # ---------------- Phase A: x^T and projections ----------------
                wo_sb = wob_pool.tile([128, DQT, D], BF16, tag="wo")
                with (
                    tc.tile_pool(name="wqkv", bufs=1) as wpool,
                    tc.tile_pool(name="xa", bufs=4) as xa_pool,
                    tc.tile_pool(name="xt", bufs=1) as xt_pool,
                    tc.tile_pool(name="pxt", bufs=2, space="PSUM") as pxt_pool,
                    tc.tile_pool(name="pproj", bufs=2, space="PSUM") as pproj_pool,
                ):
                    wq_sb = wpool.tile([128, DIN, DQ], F32R, tag="wq")
                    wk_sb = wpool.tile([128, DIN, DQ], F32R, tag="wk")
                    wv_sb = wpool.tile([128, DIN, DQ], F32R, tag="wv")
                    # weights ride their own DMA queue so the SP queue is
                    # free for x loads: phase A starts immediately
                    w_eng = getattr(nc, {"gpsimd": "gpsimd", "sync": "sync",
                                         "scalar": "scalar"}[w_engine])
                    w_eng.dma_start(
                        wq_sb[:], wq_d[:].rearrange("(c p) d -> p c d", p=128)
                    )
                    w_eng.dma_start(
                        wk_sb[:], wk_d[:].rearrange("(c p) d -> p c d", p=128)
                    )
                    w_eng.dma_start(
                        wv_sb[:], wv_d[:].rearrange("(c p) d -> p c d", p=128)
                    )
                    w_eng.dma_start(
                        wo_sb[:], wo_d[:].rearrange("(c p) d -> p c d", p=128)
                    )

                    for s in range(NS):
                        xa = []
                        for ts in range(TSUB):
                            t = xa_pool.tile([128, D], F32, tag="xa")
                            nc.sync.dma_start(
                                t[:],
                                x_d[s * SL + ts * 128 : s * SL + (ts + 1) * 128, :],
                            )
                            xa.append(t)
                        xt = xt_pool.tile([128, DIN, SL], F32, tag="xt")
                        for dc in range(0, DIN, 2):
                            pxt = pxt_pool.tile([128, 2, SL], F32, tag="pxt")
                            for j in range(2):
                                for ts in range(TSUB):
                                    nc.tensor.transpose(
                                        pxt[:, j, ts * 128 : (ts + 1) * 128],
                                        xa[ts][:, (dc + j) * 128 : (dc + j + 1) * 128],
                                        ident[:],
                                    )
                            # fp32r consumers need a rounding producer -> ACT
                            nc.scalar.activation(
                                xt[:, dc : dc + 2, :].bitcast(F32R), pxt[:], COPY
                            )

                        # Q^T and K^T (feature-major, bf16)
                        for w_sb, dst, b_sb in (
                            (wq_sb, qt, bq_sb),
                            (wk_sb, kt, bk_sb),
                        ):
                            for i in range(DQT):
                                pq = pproj_pool.tile([128, SL], F32, tag="pproj")
                                for dc in range(DIN):
                                    nc.tensor.matmul(
                                        pq[:],
                                        w_sb[:, dc, i * 128 : (i + 1) * 128],
                                        xt[:, dc, :].bitcast(F32R),
                                        start=(dc == 0),
                                        stop=(dc == DIN - 1),
                                    )
                                nc.scalar.activation(
                                    dst[:, i, s * SL : (s + 1) * SL],
                                    pq[:],
                                    IDENT,
                                    bias=b_sb[:, i : i + 1],
                                )

                        # V (token-major) with bias row
                        for ts in range(TSUB):
                            pv = pproj_pool.tile([128, DQ], F32, tag="pproj")
                            for dc in range(DIN):
                                nc.tensor.matmul(
                                    pv[:, 0:DQ],
                                    xt[:, dc, ts * 128 : (ts + 1) * 128].bitcast(F32R),
                                    wv_sb[:, dc, :],
                                    start=(dc == 0),
                                    stop=(dc == DIN - 1) and not with_bias,
                                )
                            if with_bias:
                                nc.tensor.matmul(
                                    pv[:, 0:DQ], ones_k[:], bv_sb[:],
                                    start=False, stop=True,
                                )
                            tci = s * TSUB + ts
                            nc.vector.tensor_copy(
                                vt[:, tci, :, 0:hd],
                                pv[:, 0:DQ].rearrange("p (h c) -> p h c", c=hd),
                            )

                # ---------------- Phase B: attention + O projection ----------
                with (
                    tc.tile_pool(name="att", bufs=4) as att_pool,
                    tc.tile_pool(name="attoT", bufs=2) as attoT_pool,
                    tc.tile_pool(name="outsb", bufs=2) as out_pool,
                    tc.tile_pool(name="recip", bufs=2) as recip_pool,
                    tc.tile_pool(name="pscore", bufs=2, space="PSUM") as pscore_pool,
                    tc.tile_pool(name="pav", bufs=2, space="PSUM") as pav_pool,
                    tc.tile_pool(name="pout", bufs=2, space="PSUM") as pout_pool,
                ):
                    # deferred-op queue: AV matmuls, head epilogues, and the
                    # previous slab's O-projection trail the score/exp stream
                    # so the PE never waits on the exp/normalize that feeds
                    # the next matmul
                    stream = []

                    def flush(keep):
                        while len(stream) > keep:
                            op = stream.pop(0)
                            op[0](*op[1:])

                    def emit_av(pav_ap, vt_ap, att_ap, start, stop):
                        nc.tensor.matmul(pav_ap, vt_ap, att_ap,
                                         start=start, stop=stop)

                    def emit_norm(pav, attoT, ci):
                        # normalize both heads of the pair by their ones-column
                        # rows: one PE outer product broadcasts 1/l of parity p
                        # onto rows [p*hd, (p+1)*hd)
                        rv_t = rv[ci % 2]
                        with nc.allow_low_precision(
                            reason="bf16 1/l feeds a bf16 broadcast matmul"
                        ):
                            for p in range(HPP):
                                nc.vector.reciprocal(
                                    rv_t[p * hd : p * hd + 1, :],
                                    pav[p][hd : hd + 1, :],
                                )
                        pbc = pout_pool.tile([128, SL], F32, tag="pout")
                        nc.tensor.matmul(pbc[:], ones2v[:], rv_t[:])
                        bc_sb = recip_pool.tile([128, SL], F32, tag="bc_sb")
                        nc.vector.tensor_copy(bc_sb[:], pbc[:])
                        for p in range(HPP):
                            nc.vector.tensor_mul(
                                attoT[p * hd : (p + 1) * hd, ci, :],
                                pav[p][0:hd, :],
                                bc_sb[p * hd : (p + 1) * hd, :],
                            )

                    def emit_oproj(attoT, qs, ts):
                        osb = out_pool.tile([128, D], F32, tag="outsb")
                        for ob in range(D // SL):
                            po = pout_pool.tile([128, SL], F32, tag="pout")
                            for c in range(DQT):
                                nc.tensor.matmul(
                                    po[:],
                                    attoT[:, c, ts * 128 : (ts + 1) * 128],
                                    wo_sb[:, c, ob * SL : (ob + 1) * SL],
                                    start=(c == 0),
                                    stop=(c == DQT - 1),
                                )
                            nc.vector.tensor_copy(osb[:, ob * SL : (ob + 1) * SL], po[:])
                        nc.sync.dma_start(
                            out_d[qs * SL + ts * 128 : qs * SL + (ts + 1) * 128, :],
                            osb[:],
                        )

                    for qs in range(NS):
                        attoT = attoT_pool.tile([128, DQT, SL], BF16, tag="attoT")
                        nkb = (qs + 1) * TSUB
                        for ci in range(DQT):
                            # the two heads sharing feature tile ci, at
                            # partition offsets 0 and 64: their K=64 score
                            # matmuls use disjoint PE row groups and are
                            # emitted adjacently for concurrent dispatch
                            qmv = [
                                qt[p * hd : (p + 1) * hd, ci,
                                   qs * SL : (qs + 1) * SL]
                                for p in range(HPP)
                            ]
                            kslc = [
                                (lambda kb, p=p: kt[p * hd : (p + 1) * hd, ci,
                                                    kb * 128 : (kb + 1) * 128])
                                for p in range(HPP)
                            ]
                            pav = [pav_pool.tile([hd + 1, SL], F32, tag="pav",
                                                 name=f"pav{p}")
                                   for p in range(HPP)]

                            # off-diagonal key blocks: full width; one pscore
                            # tile holds both parities, one exp covers both
                            for kb in range(qs * TSUB):
                                ps = pscore_pool.tile([128, HPP, SL], F32, tag="ps")
                                for p in range(HPP):
                                    nc.tensor.matmul(
                                        ps[:, p, :], kslc[p](kb), qmv[p],
                                        start=True, stop=True,
                                    )
                                att = att_pool.tile([128, HPP, SL], BF16, tag="att")
                                nc.scalar.activation(att[:], ps[:], EXP)
                                for p in range(HPP):
                                    stream.append(
                                        (emit_av, pav[p][:], vt[:, kb, ci * HPP + p, :],
                                         att[:, p, :], kb == 0, False)
                                    )
                                    flush(flush_w)

                            # diagonal key blocks: one score matmul over the
                            # unmasked columns >= j*128; after exp, DVE zeroes
                            # the upper triangle of the 128-wide window with a
                            # 0/1 bf16 mask (no PE mask matmul, no LDWEIGHTS
                            # switch away from kt)
                            for j in range(TSUB):
                                kb = qs * TSUB + j
                                lo = j * 128
                                hi = lo + 128
                                ps = pscore_pool.tile([128, HPP, SL], F32, tag="ps")
                                for p in range(HPP):
                                    nc.tensor.matmul(
                                        ps[:, p, lo:SL], kslc[p](kb),
                                        qmv[p][:, lo:SL],
                                        start=True, stop=True,
                                    )
                                att = att_pool.tile([128, HPP, SL], BF16, tag="att")
                                nc.scalar.activation(
                                    att[:, :, lo:SL], ps[:, :, lo:SL], EXP
                                )
                                nc.vector.tensor_mul(
                                    att[:, :, lo:hi], att[:, :, lo:hi], trib_sb[:]
                                )
                                for p in range(HPP):
                                    stream.append(
                                        (emit_av, pav[p][:, lo:SL],
                                         vt[:, kb, ci * HPP + p, :],
                                         att[:, p, lo:SL], kb == 0, kb == nkb - 1)
                                    )
                                    flush(flush_w)

                            stream.append((emit_norm, pav, attoT, ci))

                        # O projection for this slab, deferred into the next
                        # slab's score stream (partial over the head shard)
                        for ts in range(TSUB):
                            stream.append((emit_oproj, attoT, qs, ts))
                        if not oproj_defer:
                            flush(0)

                    flush(0)

            for _rep in range(repeat):
                one_pass()

    nc.compile()
    return nc


_PROGRAMS = {}


def _get_program(with_bias=True):
    if with_bias not in _PROGRAMS:
        _PROGRAMS[with_bias] = build_program(with_bias=with_bias)
    return _PROGRAMS[with_bias]


def make_masks():
    """0/1 keep-mask for the diagonal window, stacked for both parities."""
    col = np.arange(128)[None, :]
    p = np.arange(128)[:, None]
    trib = np.where(col < p, 0.0, 1.0).astype(ml_dtypes.bfloat16)
    return np.ascontiguousarray(np.stack([trib, trib], axis=1))


def _shard_inputs(x, Wq, bq, Wk, bk, Wv, bv, Wo, bo):
    """Build the 8 per-core input maps."""
    HIDDEN = Wq.shape[0]
    M = 2                     # head groups (tensor-parallel degree per batch)
    DQ = HIDDEN // M
    hd = 64
    trib = make_masks()
    s = np.float32(1.0 / np.sqrt(hd))
    in_maps = []
    for c in range(N_CORES):
        b = c // M
        g = c % M
        cols = slice(g * DQ, (g + 1) * DQ)
        in_maps.append(
            {
                "x": np.ascontiguousarray(x[b]),
                "trib": trib,
                "wq": round_f32r(Wq[:, cols] * s),
                "wk": round_f32r(Wk[:, cols]),
                "wv": round_f32r(Wv[:, cols]),
                "wo": np.ascontiguousarray(Wo[cols, :]).astype(ml_dtypes.bfloat16),
                "bq": np.ascontiguousarray(bq[cols] * s),
                "bk": np.ascontiguousarray(bk[cols]),
                "bv": round_f32r(bv[cols])[None, :],
            }
        )
    return in_maps


def kernel(**inputs) -> np.ndarray:
    x = np.asarray(inputs["x"], dtype=np.float32)
    B, L, D = x.shape
    with_bias = any(
        np.any(np.asarray(inputs[k])) for k in ("bq", "bk", "bv")
    )
    nc = _get_program(with_bias)
    in_maps = _shard_inputs(
        x,
        np.asarray(inputs["Wq"], np.float32), np.asarray(inputs["bq"], np.float32),
        np.asarray(inputs["Wk"], np.float32), np.asarray(inputs["bk"], np.float32),
        np.asarray(inputs["Wv"], np.float32), np.asarray(inputs["bv"], np.float32),
        np.asarray(inputs["Wo"], np.float32), np.asarray(inputs["bo"], np.float32),
    )
    res = run_bass_kernel_spmd(nc, in_maps, list(range(N_CORES)))
    bo = np.asarray(inputs["bo"], np.float32)
    out = np.empty((B, L, D), np.float32)
    for b in range(B):
        out[b] = res.results[2 * b]["out"] + res.results[2 * b + 1]["out"] + bo
    return out


# revision 33
# speedup vs baseline: 2.1750x; 2.1750x over previous
"""Tensor-parallel causal MHA kernel for 8 Trainium2 NeuronCores.

Problem: B=4, L=2048, D=1024, H=16 heads (hd=64), f32, causal softmax.

Sharding: batch-DP x head-TP. Core c handles batch b=c//2 and head group
g=c%2 (8 heads = 512 feature dims). Each core computes its QKV column
shard, attention for its 8 heads over its batch, and a row-shard partial
O-projection. Host sums the two partials per batch and adds bo.

Per-core pipeline (feature-transposed layout to avoid all transposes of
intermediates):
  x^T (PE transpose)  ->  Q^T/K^T = Wq/Wk^T chunks @ x^T  (feature-major,
                          stored bf16; Wq is pre-scaled by 1/sqrt(hd))
                          V       = x^T chunks @ Wv       (token-major,
                                    with a ones column per head for the
                                    softmax denominator)
  scores^T[k,q] = K^T.T @ Q^T  per head-pair as adjacent K=64 bf16
                  matmuls at partition offsets 0/64 (disjoint PE row
                  groups -> they dispatch concurrently on HW)
  diagonal key blocks: one score matmul over the causally-live columns
                  >= j*128; after exp, DVE zeroes the upper triangle of
                  the 128-wide window with a 0/1 bf16 mask (no PE mask
                  matmul, no LDWEIGHTS switch away from kt)
  att = exp(scores^T)  (no max subtraction: scores are O(5) bounded)
  AV^T[d+1,q] = V'.T @ att  accumulated over key blocks; row d is the
                softmax denominator l[q]
  normalize: att_out^T = AV^T[0:64] * broadcast(1/l); both heads of a
                pair share one PE ones-outer-product broadcast; DVE
                writes attoT (bf16) directly
  out[t,o] += att_out^T chunks.T @ Wo chunks  (bf16; partial, host adds
                the core pairs)
  the O-projection is deferred into the next slab's score stream so PE
  never stalls on the last head's normalize chain.

Weights stream in on the gpsimd SWDGE queue, x/out on the SP queue, so
phase A starts ~1.6us in instead of waiting for 19us of weight DMAs.
There are no SBUF->SBUF staging DMAs anywhere on the critical path (the
previous version staged Q, 1/l, and att_out through bit-copy DMAs).

QKV projections run in fp32r (12-bit mantissa, full PE rate; the walrus
verifier requires fp32r matmul inputs to come from rounding producers,
so x^T tiles are written by ACT with an fp32r view). The attention
score/AV path and the O-projection run in bf16 (same PE rate, full rate
even for narrow free dims, fp32 PSUM accumulation).
"""

import sys

if "/opt/trn_rl_repo" not in sys.path:
    sys.path.insert(0, "/opt/trn_rl_repo")

import ml_dtypes
import numpy as np

import concourse.bass as bass
import concourse.tile as tile
from concourse import bacc, mybir
from concourse.bass_utils import run_bass_kernel_spmd
from concourse.masks import make_identity

F32 = mybir.dt.float32
F32R = mybir.dt.float32r
BF16 = mybir.dt.bfloat16
EXP = mybir.ActivationFunctionType.Exp
IDENT = mybir.ActivationFunctionType.Identity
COPY = mybir.ActivationFunctionType.Copy

N_CORES = 8


def round_f32r(a: np.ndarray) -> np.ndarray:
    """Round fp32 values to the fp32r grid (12-bit mantissa, round half up)."""
    u = np.ascontiguousarray(a, dtype=np.float32).view(np.uint32)
    r = ((u.astype(np.uint64) + 0x800) & 0xFFFFF000).astype(np.uint32)
    return r.view(np.float32)


def build_program(L=2048, D=1024, HPC=8, hd=64, repeat=1, with_bias=True,
                  w_engine="gpsimd", flush_w=4, oproj_defer=True):
    """Build the per-core SPMD Bass program. Returns the compiled Bacc.

    repeat>1 duplicates the whole pipeline (for timing: the wall-clock delta
    between repeat=K and repeat=1 isolates on-device execution time).
    """
    DQ = HPC * hd                # per-core head dims (columns of the shard)
    SL = 512                     # slab width (queries per attention slab)
    NS = L // SL                 # slabs
    TSUB = SL // 128             # 128-row tiles per slab
    DIN = D // 128               # contraction chunks
    DQT = DQ // 128              # 128-dim tiles of the shard
    HPP = 128 // hd              # heads per 128-dim tile (2)
    VW = hd + 1                  # V' width per head (ones column appended)

    nc = bacc.Bacc("TRN2", target_bir_lowering=False, debug=False)

    x_d = nc.dram_tensor("x", [L, D], F32, kind="ExternalInput")
    wq_d = nc.dram_tensor("wq", [D, DQ], F32R, kind="ExternalInput")  # pre-scaled
    wk_d = nc.dram_tensor("wk", [D, DQ], F32R, kind="ExternalInput")
    wv_d = nc.dram_tensor("wv", [D, DQ], F32R, kind="ExternalInput")
    wo_d = nc.dram_tensor("wo", [DQ, D], BF16, kind="ExternalInput")
    bq_d = nc.dram_tensor("bq", [DQ], F32, kind="ExternalInput")   # pre-scaled
    bk_d = nc.dram_tensor("bk", [DQ], F32, kind="ExternalInput")
    bv_d = nc.dram_tensor("bv", [1, DQ], F32R, kind="ExternalInput")
    # 0/1 keep-mask (both parities) for the 128-wide diagonal window
    trib_d = nc.dram_tensor("trib", [128, 2, 128], BF16, kind="ExternalInput")
    out_d = nc.dram_tensor("out", [L, D], F32, kind="ExternalOutput")

    with tile.TileContext(nc) as tc:
        with (
            tc.tile_pool(name="persist", bufs=1) as persist,
            tc.tile_pool(name="consts", bufs=1) as consts,
        ):
            # persistent activations
            qt = persist.tile([128, DQT, L], BF16, tag="qt")
            kt = persist.tile([128, DQT, L], BF16, tag="kt")
            vt = persist.tile([128, L // 128, HPC, VW], BF16, tag="vt")

            ident = consts.tile([128, 128], F32, tag="ident")
            ones_sc = consts.tile([128, 128], F32, tag="ones_sc")
            # ones2v[p*hd, r] = 1 iff head-parity p owns output row r; the
            # 1/l rows of the two parities live at partitions 0 and hd of
            # rv[], so one K=65 outer product broadcasts both
            ones2v = consts.tile([hd + 1, 128], BF16, tag="ones2v")
            rv = [consts.tile([hd + 1, SL], BF16, tag=f"recipv{i}",
                              name=f"recipv{i}") for i in range(2)]
            bq_sb = consts.tile([128, DQT], F32, tag="bq")
            bk_sb = consts.tile([128, DQT], F32, tag="bk")
            trib_sb = consts.tile([128, HPP, 128], BF16, tag="trib")

            make_identity(nc, ident[:])
            nc.gpsimd.memset(ones_sc[:], 1.0)
            nc.gpsimd.memset(ones2v[:], 0.0)
            for i in range(2):
                nc.gpsimd.memset(rv[i][:], 0.0)
            for p in range(HPP):
                nc.scalar.activation(
                    ones2v[p * hd : p * hd + 1, p * hd : (p + 1) * hd],
                    ones_sc[0:1, 0:hd],
                    COPY,
                )
            nc.scalar.activation(
                vt[:, :, :, hd],
                ones_sc[:].rearrange("p (a b) -> p a b", a=L // 128)[:, :, 0:HPC],
                COPY,
            )
            nc.sync.dma_start(bq_sb[:], bq_d[:].rearrange("(c p) -> p c", p=128))
            nc.sync.dma_start(bk_sb[:], bk_d[:].rearrange("(c p) -> p c", p=128))
            nc.sync.dma_start(trib_sb[:], trib_d[:])
            if with_bias:
                ones_k = consts.tile([1, 128], F32R, tag="ones_k")
                bv_sb = consts.tile([1, DQ], F32R, tag="bv")
                nc.scalar.activation(ones_k[:], ones_sc[0:1, :], COPY)
                nc.sync.dma_start(bv_sb[:], bv_d[:])

            def one_pass():
              with tc.tile_pool(name="wob", bufs=1) as wob_pool:
                # ---------------- Phase A: x^T and projections ----------------
                wo_sb = wob_pool.tile([128, DQT, D], BF16, tag="wo")
                with (
                    tc.tile_pool(name="wqkv", bufs=1) as wpool,
                    tc.tile_pool(name="xa", bufs=4) as xa_pool,
                    tc.tile_pool(name="xt", bufs=1) as xt_pool,
                    tc.tile_pool(name="pxt", bufs=2, space="PSUM") as pxt_pool,
                    tc.tile_pool(name="pproj", bufs=2, space="PSUM") as pproj_pool,
                ):
                    wq_sb = wpool.tile([128, DIN, DQ], F32R, tag="wq")
                    wk_sb = wpool.tile([128, DIN, DQ], F32R, tag="wk")
                    wv_sb = wpool.tile([128, DIN, DQ], F32R, tag="wv")
                    # weights ride their own DMA queue so the SP queue is
                    # free for x loads: phase A starts immediately
                    w_eng = getattr(nc, {"gpsimd": "gpsimd", "sync": "sync",
                                         "scalar": "scalar"}[w_engine])
                    w_eng.dma_start(
                        wq_sb[:], wq_d[:].rearrange("(c p) d -> p c d", p=128)
                    )
                    w_eng.dma_start(
                        wk_sb[:], wk_d[:].rearrange("(c p) d -> p c d", p=128)
                    )
                    w_eng.dma_start(
                        wv_sb[:], wv_d[:].rearrange("(c p) d -> p c d", p=128)
                    )
                    w_eng.dma_start(
                        wo_sb[:], wo_d[:].rearrange("(c p) d -> p c d", p=128)
                    )

                    for s in range(NS):
                        xa = []
                        for ts in range(TSUB):
                            t = xa_pool.tile([128, D], F32, tag="xa")
                            nc.sync.dma_start(
                                t[:],
                                x_d[s * SL + ts * 128 : s * SL + (ts + 1) * 128, :],
                            )
                            xa.append(t)
                        xt = xt_pool.tile([128, DIN, SL], F32, tag="xt")
                        for dc in range(0, DIN, 2):
                            pxt = pxt_pool.tile([128, 2, SL], F32, tag="pxt")
                            for j in range(2):
                                for ts in range(TSUB):
                                    nc.tensor.transpose(
                                        pxt[:, j, ts * 128 : (ts + 1) * 128],
                                        xa[ts][:, (dc + j) * 128 : (dc + j + 1) * 128],
                                        ident[:],
                                    )
                            # fp32r consumers need a rounding producer -> ACT
                            nc.scalar.activation(
                                xt[:, dc : dc + 2, :].bitcast(F32R), pxt[:], COPY
                            )

                        # Q^T and K^T (feature-major, bf16)
                        for w_sb, dst, b_sb in (
                            (wq_sb, qt, bq_sb),
                            (wk_sb, kt, bk_sb),
                        ):
                            for i in range(DQT):
                                pq = pproj_pool.tile([128, SL], F32, tag="pproj")
                                for dc in range(DIN):
                                    nc.tensor.matmul(
                                        pq[:],
                                        w_sb[:, dc, i * 128 : (i + 1) * 128],
                                        xt[:, dc, :].bitcast(F32R),
                                        start=(dc == 0),
                                        stop=(dc == DIN - 1),
                                    )
                                nc.scalar.activation(
                                    dst[:, i, s * SL : (s + 1) * SL],
                                    pq[:],
                                    IDENT,
                                    bias=b_sb[:, i : i + 1],
                                )

                        # V (token-major) with bias row
                        for ts in range(TSUB):
                            pv = pproj_pool.tile([128, DQ], F32, tag="pproj")
                            for dc in range(DIN):
                                nc.tensor.matmul(
                                    pv[:, 0:DQ],
                                    xt[:, dc, ts * 128 : (ts + 1) * 128].bitcast(F32R),
                                    wv_sb[:, dc, :],
                                    start=(dc == 0),
                                    stop=(dc == DIN - 1) and not with_bias,
                                )
                            if with_bias:
                                nc.tensor.matmul(
                                    pv[:, 0:DQ], ones_k[:], bv_sb[:],
                                    start=False, stop=True,
                                )
                            tci = s * TSUB + ts
                            nc.vector.tensor_copy(
                                vt[:, tci, :, 0:hd],
                                pv[:, 0:DQ].rearrange("p (h c) -> p h c", c=hd),
                            )

                # ---------------- Phase B: attention + O projection ----------
                with (
                    tc.tile_pool(name="att", bufs=4) as att_pool,
                    tc.tile_pool(name="attoT", bufs=2) as attoT_pool,
                    tc.tile_pool(name="outsb", bufs=2) as out_pool,
                    tc.tile_pool(name="recip", bufs=2) as recip_pool,
                    tc.tile_pool(name="pscore", bufs=2, space="PSUM") as pscore_pool,
                    tc.tile_pool(name="pav", bufs=2, space="PSUM") as pav_pool,
                    tc.tile_pool(name="pout", bufs=2, space="PSUM") as pout_pool,
                ):
                    # deferred-op queue: AV matmuls, head epilogues, and the
                    # previous slab's O-projection trail the score/exp stream
                    # so the PE never waits on the exp/normalize that feeds
                    # the next matmul
                    stream = []

                    def flush(keep):
                        while len(stream) > keep:
                            op = stream.pop(0)
                            op[0](*op[1:])

                    def emit_av(pav_ap, vt_ap, att_ap, start, stop):
                        nc.tensor.matmul(pav_ap, vt_ap, att_ap,
                                         start=start, stop=stop)

                    def emit_norm(pav, attoT, ci):
                        # normalize both heads of the pair by their ones-column
                        # rows: one PE outer product broadcasts 1/l of parity p
                        # onto rows [p*hd, (p+1)*hd)
                        rv_t = rv[ci % 2]
                        with nc.allow_low_precision(
                            reason="bf16 1/l feeds a bf16 broadcast matmul"
                        ):
                            for p in range(HPP):
                                nc.vector.reciprocal(
                                    rv_t[p * hd : p * hd + 1, :],
                                    pav[p][hd : hd + 1, :],
                                )
                        pbc = pout_pool.tile([128, SL], F32, tag="pout")
                        nc.tensor.matmul(pbc[:], ones2v[:], rv_t[:])
                        bc_sb = recip_pool.tile([128, SL], F32, tag="bc_sb")
                        nc.vector.tensor_copy(bc_sb[:], pbc[:])
                        for p in range(HPP):
                            nc.vector.tensor_mul(
                                attoT[p * hd : (p + 1) * hd, ci, :],
                                pav[p][0:hd, :],
                                bc_sb[p * hd : (p + 1) * hd, :],
                            )

                    def emit_oproj(attoT, qs, ts):
                        osb = out_pool.tile([128, D], F32, tag="outsb")
                        for ob in range(D // SL):
                            po = pout_pool.tile([128, SL], F32, tag="pout")
                            for c in range(DQT):
                                nc.tensor.matmul(
                                    po[:],
                                    attoT[:, c, ts * 128 : (ts + 1) * 128],
                                    wo_sb[:, c, ob * SL : (ob + 1) * SL],
                                    start=(c == 0),
                                    stop=(c == DQT - 1),
                                )
                            nc.vector.tensor_copy(osb[:, ob * SL : (ob + 1) * SL], po[:])
                        nc.sync.dma_start(
                            out_d[qs * SL + ts * 128 : qs * SL + (ts + 1) * 128, :],
                            osb[:],
                        )

                    for qs in range(NS):
                        attoT = attoT_pool.tile([128, DQT, SL], BF16, tag="attoT")
                        nkb = (qs + 1) * TSUB
                        for ci in range(DQT):
                            # the two heads sharing feature tile ci, at
                            # partition offsets 0 and 64: their K=64 score
                            # matmuls use disjoint PE row groups and are
                            # emitted adjacently for concurrent dispatch
                            qmv = [
                                qt[p * hd : (p + 1) * hd, ci,
                                   qs * SL : (qs + 1) * SL]
                                for p in range(HPP)
                            ]
                            kslc = [
                                (lambda kb, p=p: kt[p * hd : (p + 1) * hd, ci,
                                                    kb * 128 : (kb + 1) * 128])
                                for p in range(HPP)
                            ]
                            pav = [pav_pool.tile([hd + 1, SL], F32, tag="pav",
                                                 name=f"pav{p}")
                                   for p in range(HPP)]

                            # off-diagonal key blocks: full width; one pscore
                            # tile holds both parities, one exp covers both
                            for kb in range(qs * TSUB):
                                ps = pscore_pool.tile([128, HPP, SL], F32, tag="ps")
                                for p in range(HPP):
                                    nc.tensor.matmul(
                                        ps[:, p, :], kslc[p](kb), qmv[p],
                                        start=True, stop=True,
                                    )
                                att = att_pool.tile([128, HPP, SL], BF16, tag="att")
                                nc.scalar.activation(att[:], ps[:], EXP)
                                for p in range(HPP):
                                    stream.append(
                                        (emit_av, pav[p][:], vt[:, kb, ci * HPP + p, :],
                                         att[:, p, :], kb == 0, False)
                                    )
                                    flush(flush_w)

                            # diagonal key blocks: one score matmul over the
                            # unmasked columns >= j*128; after exp, DVE zeroes
                            # the upper triangle of the 128-wide window with a
                            # 0/1 bf16 mask (no PE mask matmul, no LDWEIGHTS
                            # switch away from kt)
                            for j in range(TSUB):
                                kb = qs * TSUB + j
                                lo = j * 128
                                hi = lo + 128
                                ps = pscore_pool.tile([128, HPP, SL], F32, tag="ps")
                                for p in range(HPP):
                                    nc.tensor.matmul(
                                        ps[:, p, lo:SL], kslc[p](kb),
                                        qmv[p][:, lo:SL],
                                        start=True, stop=True,
                                    )
                                att = att_pool.tile([128, HPP, SL], BF16, tag="att")
                                nc.scalar.activation(
                                    att[:, :, lo:SL], ps[:, :, lo:SL], EXP
                                )
                                nc.vector.tensor_mul(
                                    att[:, :, lo:hi], att[:, :, lo:hi], trib_sb[:]
                                )
                                for p in range(HPP):
                                    stream.append(
                                        (emit_av, pav[p][:, lo:SL],
                                         vt[:, kb, ci * HPP + p, :],
                                         att[:, p, lo:SL], kb == 0, kb == nkb - 1)
                                    )
                                    flush(flush_w)

                            stream.append((emit_norm, pav, attoT, ci))

                        # O projection for this slab, deferred into the next
                        # slab's score stream (partial over the head shard)
                        for ts in range(TSUB):
                            stream.append((emit_oproj, attoT, qs, ts))
                        if not oproj_defer:
                            flush(0)

                    flush(0)

            for _rep in range(repeat):
                one_pass()

    nc.compile()
    return nc


_PROGRAMS = {}


def _get_program(with_bias=True):
    if with_bias not in _PROGRAMS:
        _PROGRAMS[with_bias] = build_program(with_bias=with_bias)
    return _PROGRAMS[with_bias]


def make_masks():
    """0/1 keep-mask for the diagonal window, stacked for both parities."""
    col = np.arange(128)[None, :]
    p = np.arange(128)[:, None]
    trib = np.where(col < p, 0.0, 1.0).astype(ml_dtypes.bfloat16)
    return np.ascontiguousarray(np.stack([trib, trib], axis=1))


def _shard_inputs(x, Wq, bq, Wk, bk, Wv, bv, Wo, bo):
    """Build the 8 per-core input maps."""
    HIDDEN = Wq.shape[0]
    M = 2                     # head groups (tensor-parallel degree per batch)
    DQ = HIDDEN // M
    hd = 64
    trib = make_masks()
    s = np.float32(1.0 / np.sqrt(hd))
    groups = []
    for g in range(M):
        cols = slice(g * DQ, (g + 1) * DQ)
        groups.append(
            {
                "trib": trib,
                "wq": round_f32r(Wq[:, cols] * s),
                "wk": round_f32r(Wk[:, cols]),
                "wv": round_f32r(Wv[:, cols]),
                "wo": np.ascontiguousarray(Wo[cols, :]).astype(ml_dtypes.bfloat16),
                "bq": np.ascontiguousarray(bq[cols] * s),
                "bk": np.ascontiguousarray(bk[cols]),
                "bv": round_f32r(bv[cols])[None, :],
            }
        )
    return [{"x": np.ascontiguousarray(x[c // M]), **groups[c % M]}
            for c in range(N_CORES)]


def kernel(**inputs) -> np.ndarray:
    x = np.asarray(inputs["x"], dtype=np.float32)
    B, L, D = x.shape
    with_bias = any(
        np.any(np.asarray(inputs[k])) for k in ("bq", "bk", "bv")
    )
    nc = _get_program(with_bias)
    in_maps = _shard_inputs(
        x,
        np.asarray(inputs["Wq"], np.float32), np.asarray(inputs["bq"], np.float32),
        np.asarray(inputs["Wk"], np.float32), np.asarray(inputs["bk"], np.float32),
        np.asarray(inputs["Wv"], np.float32), np.asarray(inputs["bv"], np.float32),
        np.asarray(inputs["Wo"], np.float32), np.asarray(inputs["bo"], np.float32),
    )
    res = run_bass_kernel_spmd(nc, in_maps, list(range(N_CORES)))
    bo = np.asarray(inputs["bo"], np.float32)
    out = np.empty((B, L, D), np.float32)
    for b in range(B):
        out[b] = res.results[2 * b]["out"] + res.results[2 * b + 1]["out"] + bo
    return out
